# revision 8
# baseline (speedup 1.0000x reference)
"""BitLinear forward on 8 Trainium2 NeuronCores.

out = (x_q @ w_q) * (beta * gamma)
  a      = mean(weight);  w_q = sign(weight - a)
  gamma  = max|x| per row; x_q = clip(x/(gamma+eps), -(1-eps), 1-eps)
  beta   = max|weight|

Sharding: data-parallel over rows of x (N=32768 -> 4096 rows/core),
weight (1024x1024) replicated; per-core scalar stats are computed
redundantly so no collectives are needed.

Kernel math note: since QB == 1, (x_q @ w_q)*beta*gamma equals
(x @ w_q) * beta*gamma/(gamma+eps) up to the +-(1-eps) clip, which only
affects the row-max element by <=1e-5 relative -- far below the bf16
rounding used for the matmul. So the kernel never materializes x_q; it
feeds bf16(x) to the tensor engine and folds everything into the output
scale s = beta*gamma/(gamma+eps).
"""

import sys

import numpy as np

if "/opt/trn_rl_repo" not in sys.path:
    sys.path.insert(0, "/opt/trn_rl_repo")

N_CORES = 8
N_FEAT = 1024
N_OUT = 1024
P = 128
KC = N_FEAT // P  # 8 contraction chunks of 128
EPS = 1e-5

_NC_CACHE = {}
_PATCHED = False


def _split_multi_waits(nc, max_waits=1):
    """The walrus build in this image rejects instructions carrying more
    than one sync-wait ("Too many sync wait commands").  Tile's semaphore
    assignment attaches one wait per producer proc, so hoist surplus waits
    onto NOP carrier instructions inserted immediately before the waiting
    instruction on the same engine (waits execute before the instruction
    body, so this preserves semantics exactly)."""
    import bass_rust

    for fn in nc.m.functions:
        for blk in fn.blocks:
            insts = blk.instructions  # live list
            i = 0
            while i < len(insts):
                ins = insts[i]
                si = getattr(ins, "sync_info", None)
                if si is None:
                    i += 1
                    continue
                waits = list(si.on_wait)
                if len(waits) <= max_waits:
                    i += 1
                    continue
                keep = waits[:max_waits]
                surplus = waits[max_waits:]
                si.on_wait = keep
                carriers = []
                cur_list = nc.cur_bb.bb.instructions
                for j in range(0, len(surplus), max_waits):
                    nop = nc.engines[ins.engine].nop(nofuse=True)
                    nop.ins.sync_info = bass_rust.SyncInfo(
                        on_wait=surplus[j : j + max_waits], on_update=[]
                    )
                    popped = cur_list.pop()
                    assert popped is nop.ins
                    carriers.append(nop.ins)
                for k, c in enumerate(carriers):
                    insts.insert(i + k, c)
                i += len(carriers) + 1


def _patch_tile_drain():
    global _PATCHED
    if _PATCHED:
        return
    _PATCHED = True
    import concourse.tile as tile

    orig = tile.TileContext._drain_and_barrier

    def patched(self, tick_clock, wait_clock):
        orig(self, tick_clock, wait_clock)
        _split_multi_waits(self.nc)

    tile.TileContext._drain_and_barrier = patched


def _build_nc(rows_per_core: int):
    import concourse.bass as bass
    import concourse.mybir as mybir
    import concourse.tile as tile

    _patch_tile_drain()

    f32 = mybir.dt.float32
    bf16 = mybir.dt.bfloat16
    R = rows_per_core
    assert R % P == 0
    T = R // P

    nc = bass.Bass("TRN2", target_bir_lowering=False, debug=False)
    x_h = nc.declare_dram_parameter("x", [R, N_FEAT], f32, isOutput=False)
    w_h = nc.declare_dram_parameter("weight", [N_FEAT, N_OUT], f32, isOutput=False)
    o_h = nc.declare_dram_parameter("out", [R, N_OUT], f32, isOutput=True)
    # DRAM scratch for the cross-partition (mean / absmax) reductions
    scr_s = nc.dram_tensor("scr_s", [P], f32)
    scr_b = nc.dram_tensor("scr_b", [P], f32)
    scr_2 = nc.dram_tensor("scr_2", [2], f32)

    x_ap = x_h[:, :]
    o_ap = o_h[:, :]
    # weight[c*128 + p, n] -> [p, c, n]
    w_ap = w_h[:, :].rearrange("(c p) n -> p c n", p=P)

    with tile.TileContext(nc) as tc:
        with (
            tc.tile_pool(name="wpool", bufs=1) as wpool,
            tc.tile_pool(name="xpool", bufs=4) as xpool,
            tc.tile_pool(name="bpool", bufs=3) as bpool,
            tc.tile_pool(name="tpool", bufs=3) as tpool,
            tc.tile_pool(name="opool", bufs=3) as opool,
            tc.tile_pool(name="spool", bufs=8) as spool,
            tc.tile_pool(name="pspool", bufs=3, space="PSUM") as pspool,
        ):
            # ---- weight preamble ----
            w32 = wpool.tile([P, KC, N_OUT], f32, tag="w32")
            wq = wpool.tile([P, KC, N_OUT], bf16, tag="wq")
            wsum = wpool.tile([P, KC], f32, tag="wsum")
            wmax = wpool.tile([P, KC], f32, tag="wmax")
            ssum = wpool.tile([P, 1], f32, tag="ssum")
            bmax = wpool.tile([P, 1], f32, tag="bmax")
            rows_s = wpool.tile([1, P], f32, tag="rows_s")
            rows_b = wpool.tile([1, P], f32, tag="rows_b")
            pack2 = wpool.tile([1, 2], f32, tag="pack2")
            stats = wpool.tile([P, 2], f32, tag="stats")

            for c in range(KC):
                nc.sync.dma_start(out=w32[:, c, :], in_=w_ap[:, c, :])
            for c in range(KC):
                nc.vector.tensor_reduce(
                    wsum[:, c : c + 1], w32[:, c, :],
                    axis=mybir.AxisListType.X, op=mybir.AluOpType.add,
                )
                nc.vector.tensor_reduce(
                    wmax[:, c : c + 1], w32[:, c, :],
                    axis=mybir.AxisListType.X, op=mybir.AluOpType.max,
                    apply_absolute_value=True,
                )
            nc.vector.tensor_reduce(
                ssum, wsum, axis=mybir.AxisListType.X, op=mybir.AluOpType.add
            )
            nc.vector.tensor_reduce(
                bmax, wmax, axis=mybir.AxisListType.X, op=mybir.AluOpType.max
            )
            # cross-partition reduction via DRAM round-trip:
            # [128,1] -> DRAM[128] -> [1,128] -> reduce on partition 0
            nc.sync.dma_start(out=scr_s[:], in_=ssum[:, 0:1])
            nc.sync.dma_start(out=scr_b[:], in_=bmax[:, 0:1])
            nc.sync.dma_start(out=rows_s, in_=scr_s[None, :])
            nc.sync.dma_start(out=rows_b, in_=scr_b[None, :])
            # pack2 = [-mean, beta] on partition 0
            nc.vector.tensor_reduce(
                pack2[:, 0:1], rows_s, axis=mybir.AxisListType.X,
                op=mybir.AluOpType.add,
            )
            nc.vector.tensor_scalar_mul(
                pack2[:, 0:1], pack2[:, 0:1], -1.0 / float(N_FEAT * N_OUT)
            )
            nc.vector.tensor_reduce(
                pack2[:, 1:2], rows_b, axis=mybir.AxisListType.X,
                op=mybir.AluOpType.max,
            )
            # broadcast to all 128 partitions via DRAM
            nc.sync.dma_start(out=scr_2[:], in_=pack2[0:1, :])
            scr2_ap = scr_2[:]
            bcast_src = bass.AP(
                tensor=scr2_ap.tensor, offset=scr2_ap.offset,
                ap=[[0, P]] + list(scr2_ap.ap),
            )
            nc.sync.dma_start(out=stats, in_=bcast_src)
            neg_a = stats[:, 0:1]
            beta = stats[:, 1:2]

            # w_q = sign(w - a), cast to bf16 (exact +-1)
            for c in range(KC):
                nc.scalar.activation(
                    out=wq[:, c, :], in_=w32[:, c, :],
                    func=mybir.ActivationFunctionType.Sign,
                    bias=neg_a, scale=1.0,
                )

            # ---- main loop over 128-row tiles ----
            for t in range(T):
                rows = slice(t * P, (t + 1) * P)

                x32 = xpool.tile([P, N_FEAT], f32, tag="x32")
                nc.sync.dma_start(out=x32, in_=x_ap[rows, :])

                gamma = spool.tile([P, 1], f32, tag="gamma")
                nc.vector.tensor_reduce(
                    gamma, x32, axis=mybir.AxisListType.X,
                    op=mybir.AluOpType.max, apply_absolute_value=True,
                )
                # s = beta * gamma / (gamma + eps)
                ginv = spool.tile([P, 1], f32, tag="ginv")
                nc.vector.tensor_scalar_add(ginv, gamma, float(EPS))
                nc.vector.reciprocal(ginv, ginv)
                s = spool.tile([P, 1], f32, tag="s")
                nc.vector.tensor_mul(s, gamma, ginv)
                nc.vector.tensor_mul(s, s, beta)

                xb = bpool.tile([P, N_FEAT], bf16, tag="xb")
                nc.scalar.copy(xb, x32)

                # xT[p, c, r] = xb[r, c*128 + p] via xbar DMA transpose
                xT = tpool.tile([P, KC, P], bf16, tag="xT")
                nc.sync.dma_start_transpose(out=xT, in_=xb)

                ps = pspool.tile([P, N_OUT], f32, tag="ps")
                for c in range(KC):
                    for h in range(2):
                        nc.tensor.matmul(
                            ps[:, h * 512 : (h + 1) * 512],
                            xT[:, c, :],
                            wq[:, c, h * 512 : (h + 1) * 512],
                            start=(c == 0),
                            stop=(c == KC - 1),
                        )

                o = opool.tile([P, N_OUT], f32, tag="o")
                nc.scalar.activation(
                    out=o, in_=ps,
                    func=mybir.ActivationFunctionType.Copy,
                    bias=0.0, scale=s,
                )
                nc.sync.dma_start(out=o_ap[rows, :], in_=o)

    return nc


def _get_nc(rows_per_core: int):
    if rows_per_core not in _NC_CACHE:
        _NC_CACHE[rows_per_core] = _build_nc(rows_per_core)
    return _NC_CACHE[rows_per_core]


def run(x, weight, trace=False, trace_cores=None):
    """Run on 8 cores; returns (out, BassKernelResults)."""
    from concourse.bass_utils import run_bass_kernel_spmd

    x = np.ascontiguousarray(np.asarray(x, dtype=np.float32))
    weight = np.ascontiguousarray(np.asarray(weight, dtype=np.float32))
    n = x.shape[0]
    assert n % N_CORES == 0
    rpc = n // N_CORES
    nc = _get_nc(rpc)
    in_maps = [
        {"x": x[i * rpc : (i + 1) * rpc], "weight": weight} for i in range(N_CORES)
    ]
    kwargs = {}
    if trace:
        kwargs["trace"] = True
        if trace_cores is not None:
            kwargs["trace_cores"] = trace_cores
    res = run_bass_kernel_spmd(nc, in_maps, core_ids=list(range(N_CORES)), **kwargs)
    out = np.concatenate([r["out"] for r in res.results], axis=0)
    return out, res


def kernel(x, weight):
    out, _ = run(x, weight)
    return out
